# revision 55
# baseline (speedup 1.0000x reference)
"""HGAT (GRU + decayed attention + 2x HypergraphConv over 9 hypergraphs) on 8 trn2 cores.

Strategy:
  - Host: fold each hypergraph's D^-1 H B^-1 H^T into ONE dense [1152,1152]
    propagation matrix A (bf16, shipped transposed), so each conv is a single
    64x1152x1152 matmul instead of two + transposes.
  - Device (SPMD, 8 cores):
      * GRU + attention sharded over nodes (144/core); recurrent gate
        pre-activations accumulate x-projections + h-projections directly in
        PSUM (no vector add on the critical path); bf16 throughout.
      * cc1: AllGather of x@theta1 (bf16, node-major) - the only pre-conv
        collective.
      * Convs data-parallel over timesteps (core c: timestep c's 2-conv
        chain), slab-pipelined epilogues. The GLOBAL-hypergraph chain is
        row-distributed: stage 1 computes x1g rows for this core only;
        stage 2 contributes a partial sum A_G[:, rows_c] @ (x1g theta2)
        which rides the ReduceScatter.
      * cc2: single ReduceScatter combining (a) signed x2 slices from cores
        0-3 (slot0 sums to x2[1]-x2[0], slot1 to x2[3]-x2[2]), (b) global
        partial sums, (c) per-timestep scalar sums S_t.
      * Short final stage per-core on its 144 rows (difference matrix folded
        into w1 on the host).
  - Host assembles the 8 row-slices into the [1026,1] output.
"""
import numpy as np
import ml_dtypes

N, T, H, F_IN, E = 1026, 8, 64, 5, 1026
NP = 1152            # padded N and E (9 * 128)
NCORES = 8
SL = NP // NCORES    # 144 nodes per core
NCH = NP // 128      # 9 contraction chunks
BF = ml_dtypes.bfloat16

_NC_CACHE = {}
DEBUG = False

# payload chunk layout (per destination core), cols of a [64, PAYW] block
PAY_C = 2 * SL        # 288: global conv2 partial sums (144)
PAY_S = 3 * SL        # 432: per-timestep sums S (8)
PAYW = PAY_S + 8      # 440


# --------------------------------------------------------------------------
# host-side prep
# --------------------------------------------------------------------------

def _prop_matrix(idx):
    """Dense padded propagation matrix A = D^-1 H B^-1 H^T, [NP, NP] f32."""
    node = idx[0].astype(np.int64)
    edge = idx[1].astype(np.int64)
    Hm = np.bincount(node * E + edge, minlength=N * E).reshape(N, E)
    Hm = Hm.astype(np.float32)
    degn = Hm.sum(1)
    dege = Hm.sum(0)
    with np.errstate(divide="ignore"):
        Dinv = np.where(degn > 0, 1.0 / degn, 0.0).astype(np.float32)
        Binv = np.where(dege > 0, 1.0 / dege, 0.0).astype(np.float32)
    A = (Hm * Binv[None, :]) @ Hm.T * Dinv[:, None]
    Ap = np.zeros((NP, NP), np.float32)
    Ap[:N, :N] = A
    return Ap


def _host_prep(inp):
    f32 = np.float32
    price = np.asarray(inp["price_input"], f32)          # [N, T, F]
    hyp_T = np.asarray(inp["hyp_T"])                     # [T, 2, nnz]
    hyp = np.asarray(inp["hyp"])                         # [2, nnz]

    WihT = np.ascontiguousarray(np.asarray(inp["Wih"], f32).T)   # [5, 192]
    WhhT = np.ascontiguousarray(np.asarray(inp["Whh"], f32).T)   # [64, 192]
    bih = np.asarray(inp["bih"], f32)
    bhh = np.asarray(inp["bhh"], f32)
    b1 = np.asarray(inp["bias1"], f32)
    b2 = np.asarray(inp["bias2"], f32)
    bl = float(np.asarray(inp["bl"], f32)[0])

    c64f = np.zeros((64, 10), f32)
    c64f[:, 0] = bih[128:192]
    c64f[:, 1] = bhh[128:192]
    c64f[:, 2] = b1
    c64f[:, 3] = 0.2 * b1
    c64f[:, 4] = b2
    c64f[:, 5] = 0.2 * b2
    c64f[:, 6] = 1.0                      # ones64
    c64f[:, 7] = np.asarray(inp["Wl"], f32)[64:128, 0]

    c128f = np.zeros((128, 20), f32)
    c128f[:, 0] = bih[0:128] + bhh[0:128]   # combined r/z gate bias
    c128f[:, 2] = np.asarray(inp["Wl"], f32)[:, 0]
    c128f[:, 3:11] = np.arange(T - 1, -1, -1, dtype=f32)[None, :]  # delta
    c128f[0, 19] = 1.0                                             # ones row

    cb64 = np.zeros((64, 386), BF)
    cb64[:, 0:128] = WhhT[:, 0:128].astype(BF)
    cb64[:, 128:192] = WhhT[:, 128:192].astype(BF)
    cb64[:, 192:256] = np.asarray(inp["Win"], f32).astype(BF)
    cb64[:, 256:320] = np.asarray(inp["theta1"], f32).astype(BF)
    cb64[:, 320:384] = np.asarray(inp["theta2"], f32).astype(BF)
    cb64[:, 384] = np.asarray(inp["Wl"], f32)[0:64, 0].astype(BF)
    cb64[:, 385] = np.asarray(inp["Wl"], f32)[64:128, 0].astype(BF)

    cb128 = np.zeros((128, 192), BF)
    cb128[:, 0:64] = np.asarray(inp["Wout"], f32).astype(BF)
    cb128[:, 64:192] = np.eye(128, dtype=f32).astype(BF)

    # y = w1 @ z with z_t = S_{t+1} - S_t folded: y = M @ S, M[h,s] =
    # w1[h,s-1] - w1[h,s] (out-of-range terms zero). Shipped transposed.
    w1 = np.asarray(inp["w1"], f32)       # [64, 7]
    M = np.zeros((64, 8), f32)
    M[:, 1:8] += w1
    M[:, 0:7] -= w1
    # w2T for wat = (w2 @ y)^T computed as y^T @ w2^T
    w2T = np.ascontiguousarray(np.asarray(inp["w2"], f32).T)   # [64, 7]

    shared = {
        "c64f": c64f, "c128f": c128f, "cb64": cb64, "cb128": cb128,
        "cwih": np.ascontiguousarray(WihT.astype(BF)),         # [5,192] bf16
        "cMT": np.ascontiguousarray(M.T),                      # [8,64]
        "cw2T": w2T,                                           # [64,7]
        "cbl": np.array([[bl, 0.2 * bl]], f32),                # [1,2]
    }

    A_G = _prop_matrix(hyp)
    AGT = np.ascontiguousarray(A_G.T)           # [n, n'] (contraction-major)

    price_p = np.zeros((NP, T, F_IN), f32)
    price_p[:N] = price
    ae_p = np.zeros((NP,), f32)
    ae_p[:N] = np.asarray(inp["ae"], f32)[:, 0, 0]
    ab_p = np.zeros((NP,), f32)
    ab_p[:N] = np.asarray(inp["ab"], f32)[:, 0, 0]

    in_maps = []
    for c in range(NCORES):
        sl = slice(c * SL, (c + 1) * SL)
        m = dict(shared)
        m["x5"] = np.ascontiguousarray(
            price_p[sl].transpose(2, 1, 0).reshape(F_IN, T * SL).astype(BF))
        pc = np.zeros((SL, 2), f32)
        pc[:, 0] = ae_p[sl]
        pc[:, 1] = -ab_p[sl]
        m["pc144"] = pc
        # mask columns: slot0 = x2[1]-x2[0], slot1 = x2[3]-x2[2]
        oh = np.zeros((64, 11), f32)
        oh[:, c] = 1.0
        oh[:, 8] = {0: -1.0, 1: 1.0}.get(c, 0.0)
        oh[:, 9] = {2: -1.0, 3: 1.0}.get(c, 0.0)
        m["onehot"] = oh
        m["AT_L"] = np.ascontiguousarray(_prop_matrix(hyp_T[c]).T.astype(BF))
        m["AGT_col"] = np.ascontiguousarray(AGT[:, sl].astype(BF))  # [1152,144]
        m["AGT_row"] = np.ascontiguousarray(AGT[sl, :].astype(BF))  # [144,1152]
        in_maps.append(m)
    return in_maps


_IN_SPECS = [
    ("c64f", (64, 10), "f32"), ("c128f", (128, 20), "f32"),
    ("cb64", (64, 386), "bf16"), ("cb128", (128, 192), "bf16"),
    ("cwih", (F_IN, 192), "bf16"), ("cMT", (8, 64), "f32"),
    ("cw2T", (64, 7), "f32"), ("cbl", (1, 2), "f32"),
    ("x5", (F_IN, T * SL), "bf16"), ("pc144", (SL, 2), "f32"),
    ("onehot", (64, 11), "f32"),
    ("AT_L", (NP, NP), "bf16"),
    ("AGT_col", (NP, SL), "bf16"), ("AGT_row", (SL, NP), "bf16"),
]


# --------------------------------------------------------------------------
# device program
# --------------------------------------------------------------------------

def build_program(tc, A, out_ap):
    import contextlib
    import concourse.bass as bass
    import concourse.mybir as mybir

    nc = tc.nc
    F32 = mybir.dt.float32
    F32R = mybir.dt.float32r
    BF16 = mybir.dt.bfloat16
    AF = mybir.ActivationFunctionType
    ALU = mybir.AluOpType
    AX = mybir.AxisListType
    CH3 = ((0, 512), (512, 512), (1024, 128))
    groups = [list(range(NCORES))]

    stack = contextlib.ExitStack()
    CP = stack.enter_context(tc.tile_pool(name="consts", bufs=1))
    WK = stack.enter_context(tc.tile_pool(name="work", bufs=1))
    HP = stack.enter_context(tc.tile_pool(name="amat", bufs=1))
    DR = stack.enter_context(tc.tile_pool(name="dram", bufs=1, space="DRAM"))

    # ---- DRAM comm buffers ----
    cc1_in = DR.tile([SL, 64], BF16, name="cc1_in")
    cc1_out = DR.tile([NP, 64], BF16, name="cc1_out", addr_space="Shared")
    cc2_in = DR.tile([NCORES, 64, PAYW], F32, name="cc2_in")
    cc2_out = DR.tile([64, PAYW], F32, name="cc2_out")

    # ---- SBUF const tiles; all on the SP queue, small/urgent ones first so
    # they reach DMA_ENGINES before the multi-MB operand loads ----
    def load(pool, name, shape, dtype, src_ap, eng=None):
        t = pool.tile(shape, dtype, name=name)
        (eng or nc.sync).dma_start(t[:], src_ap)
        return t

    x5 = load(CP, "x5", [F_IN, T * SL], BF16, A["x5"][:])
    cwih = load(CP, "cwih", [F_IN, 192], BF16, A["cwih"][:], nc.scalar)
    c128f = load(CP, "c128f", [128, 20], F32, A["c128f"][:], nc.gpsimd)
    c64f = load(CP, "c64f", [64, 10], F32, A["c64f"][:])
    cb64 = load(CP, "cb64", [64, 386], BF16, A["cb64"][:], nc.scalar)
    cb128 = load(CP, "cb128", [128, 192], BF16, A["cb128"][:])
    cMT = load(CP, "cMT", [8, 64], F32, A["cMT"][:])
    cw2T = load(CP, "cw2T", [64, 7], F32, A["cw2T"][:])
    cbl = load(CP, "cbl", [1, 2], F32, A["cbl"][:])
    pcA = load(CP, "pcA", [128, 2], F32, A["pc144"][0:128])
    pcB = load(CP, "pcB", [16, 2], F32, A["pc144"][128:SL])
    onehot = load(CP, "onehot", [64, 11], F32, A["onehot"][:])

    # named const views
    brz = c128f[:, 0:1]           # bih+bhh for r/z gates
    Wl = c128f[:, 2:3]
    delta = c128f[:, 3:11]
    ones1x64 = c128f[0:1, 19:20].broadcast_to([1, 64])
    bih_n = c64f[:, 0:1]
    bhh_n = c64f[:, 1:2]
    b1c = c64f[:, 2:3]
    b1q = c64f[:, 3:4]
    b2c = c64f[:, 4:5]
    b2q = c64f[:, 5:6]
    ones64 = c64f[:, 6:7]
    Wl_xx = c64f[:, 7:8]
    WhhT_rz = cb64[:, 0:128]
    WhhT_n = cb64[:, 128:192]
    Win = cb64[:, 192:256]
    theta1 = cb64[:, 256:320]
    theta2 = cb64[:, 320:384]
    Wl_xg_b = cb64[:, 384:385]
    Wl_xx_b = cb64[:, 385:386]
    Wout = cb128[:, 0:64]
    identB = cb128[:, 64:192]
    identB64 = cb128[0:64, 64:128]
    WihT_rz = cwih[:, 0:128]
    WihT_n = cwih[:, 128:192]
    aeA, nabA = pcA[:, 0:1], pcA[:, 1:2]
    aeB, nabB = pcB[:, 0:1], pcB[:, 1:2]
    sgn8 = onehot[:, 0:8]
    sg0c = onehot[:, 8:9]
    sg1c = onehot[:, 9:10]

    # ---- big per-core operand loads (SP queue, after the consts) ----
    AT_L = HP.tile([128, NCH, NP], BF16, name="AT_L")
    nc.sync.dma_start(AT_L[:], A["AT_L"][:].rearrange("(k p) n -> p k n", p=128))
    AGc = HP.tile([128, NCH, SL], BF16, name="AGc")
    nc.sync.dma_start(AGc[:], A["AGT_col"][:].rearrange("(k p) n -> p k n", p=128))
    AGrA = HP.tile([128, NP], BF16, name="AGrA")
    nc.sync.dma_start(AGrA[:], A["AGT_row"][0:128, :])
    AGrB = HP.tile([16, NP], BF16, name="AGrB")
    nc.sync.dma_start(AGrB[:], A["AGT_row"][128:SL, :])

    # hoist the sigmoid/tanh activation-table load off the critical path
    dmy = WK.tile([1, 1], F32, name="dmy")
    nc.vector.memset(dmy[:], 0.0)
    dmy2 = WK.tile([1, 1], F32, name="dmy2")
    nc.scalar.activation(dmy2[:], dmy[:], AF.Sigmoid)

    # ---- persistent work tiles ----
    ctxT = WK.tile([64, T * SL], BF16, name="ctxT")        # h per step, [h,(t n)]
    ctx_nA = WK.tile([128, T, 64], BF16, name="ctx_nA")
    ctx_nB = WK.tile([16, T, 64], BF16, name="ctx_nB")
    xq1 = WK.tile([128, NCH, 64], BF16, name="xq1")        # gathered x@theta1
    x2pay = WK.tile([64, NP], F32, name="x2pay")           # signed leaky(conv2)
    pC = WK.tile([64, NP], F32, name="pC")
    S8 = WK.tile([64, 8], F32, name="S8")
    R = WK.tile([64, PAYW], F32, name="R")                 # RS result

    # ======================= GRU =======================
    # gi_n (n-gate x-projection) precomputed to SBUF; r/z gate x-projections
    # go straight into the per-step PSUM accumulators.
    PSGI_pool = tc.tile_pool(name="ps_gi", bufs=1, space="PSUM")
    PSGI = PSGI_pool.__enter__()
    gi_n_ps = PSGI.tile([64, T * SL], F32, name="gi_n_ps", tag="g2")

    with tc.tile_pool(name="ps_grz", bufs=2, space="PSUM") as PSRZ, \
         tc.tile_pool(name="ps_gn", bufs=1, space="PSUM") as PSN, \
         tc.tile_pool(name="ps_tr", bufs=2, space="PSUM") as PST, \
         tc.tile_pool(name="sb_gru", bufs=2) as SBG:
        rz_ps = []
        for t in range(2):
            ps = PSRZ.tile([128, SL], F32, name=f"rzps{t}", tag="grz")
            s = slice(t * SL, (t + 1) * SL)
            nc.tensor.matmul(ps[:], WihT_rz, x5[:, s], start=True,
                             stop=(t == 0))
            rz_ps.append(ps)
        nc.tensor.matmul(gi_n_ps[:, 0:512], WihT_n, x5[:, 0:512],
                         start=True, stop=True)
        for t in range(T):
            s = slice(t * SL, (t + 1) * SL)
            sp = slice((t - 1) * SL, t * SL)
            ps = rz_ps[t]
            if t > 0:
                nc.tensor.matmul(ps[:], WhhT_rz, ctxT[:, sp],
                                 start=False, stop=True)
            rz = SBG.tile([128, SL], BF16, name="rz", tag="rz")
            nc.scalar.activation(rz[0:64, :], ps[0:64, :], AF.Sigmoid,
                                 bias=brz[0:64, :])
            if t == 0:
                for o_, w_ in ((512, 512), (1024, 128)):
                    nc.tensor.matmul(gi_n_ps[:, o_:o_ + w_], WihT_n,
                                     x5[:, o_:o_ + w_], start=True, stop=True)
            z0 = SBG.tile([64, SL], BF16, name="z0", tag="z0")
            nc.scalar.activation(z0[:], ps[64:128, :], AF.Sigmoid,
                                 bias=brz[64:128, :])
            if t + 2 < T:
                ps3 = PSRZ.tile([128, SL], F32, name=f"rzps{t + 2}", tag="grz")
                s3 = slice((t + 2) * SL, (t + 3) * SL)
                nc.tensor.matmul(ps3[:], WihT_rz, x5[:, s3], start=True,
                                 stop=False)
                rz_ps.append(ps3)
            if t == 0:
                wn = SBG.tile([64, SL], BF16, name="wn", tag="wn")
                nc.vector.tensor_scalar(wn[:], rz[0:64, :], bhh_n, None,
                                        ALU.mult)
            else:
                gh_n = PSN.tile([64, SL], F32, name="gh_n", tag="ghn")
                nc.tensor.matmul(gh_n[:], WhhT_n, ctxT[:, sp],
                                 start=True, stop=True)
                wn = SBG.tile([64, SL], BF16, name="wn", tag="wn")
                nc.vector.scalar_tensor_tensor(wn[:], gh_n[:], bhh_n,
                                               rz[0:64, :], ALU.add, ALU.mult)
            un = SBG.tile([64, SL], BF16, name="un", tag="un")
            nc.vector.scalar_tensor_tensor(un[:], gi_n_ps[:, s], bih_n,
                                           wn[:], ALU.add, ALU.add)
            nt = SBG.tile([64, SL], BF16, name="nt", tag="nt")
            nc.scalar.activation(nt[:], un[:], AF.Tanh)
            omz = SBG.tile([64, SL], BF16, name="omz", tag="omz")
            nc.vector.tensor_scalar(omz[:], z0[:], -1.0, 1.0,
                                    ALU.mult, ALU.add)
            if t == 0:
                nc.vector.tensor_tensor(ctxT[:, s], omz[:], nt[:], ALU.mult)
            else:
                za = SBG.tile([64, SL], BF16, name="za", tag="za")
                nc.vector.tensor_tensor(za[:], z0[:], ctxT[:, sp], ALU.mult)
                bb = SBG.tile([64, SL], BF16, name="bb", tag="bb")
                nc.vector.tensor_tensor(bb[:], omz[:], nt[:], ALU.mult)
                nc.vector.tensor_tensor(ctxT[:, s], bb[:], za[:], ALU.add)
            trp = PST.tile([128, 2, 64], BF16, name="trp", tag="tr")
            nc.tensor.transpose(trp[:, 0, :], ctxT[:, t * SL:t * SL + 128],
                                identB64)
            nc.tensor.transpose(trp[0:16, 1, :],
                                ctxT[:, t * SL + 128:(t + 1) * SL], identB64)
            nc.vector.tensor_copy(ctx_nA[:, t, :], trp[:, 0, :])
            nc.vector.tensor_copy(ctx_nB[:, t, :], trp[0:16, 1, :])

    PSGI_pool.__exit__(None, None, None)

    # ======================= attention =======================
    with tc.tile_pool(name="ps_att", bufs=1, space="PSUM") as PSA, \
         tc.tile_pool(name="sb_att", bufs=1) as SBA:
        lastT = ctxT[:, 7 * SL:8 * SL]
        combT = SBA.tile([128, SL], BF16, name="combT")
        qT_ps = PSA.tile([64, SL], F32, name="qT_ps", tag="qT")
        nc.tensor.matmul(qT_ps[:], Win, lastT, start=True, stop=True)
        nc.scalar.activation(combT[64:128, :], qT_ps[:], AF.Copy)

        for nm, np_, ctx_n, ae_t, nab_t, csl in (
                ("A", 128, ctx_nA, aeA, nabA, slice(0, 128)),
                ("B", 16, ctx_nB, aeB, nabB, slice(128, SL))):
            q_ps = PSA.tile([np_, 64], F32, name=f"q_ps{nm}", tag=f"q{nm}")
            nc.tensor.matmul(q_ps[:], lastT[:, csl], Win,
                             start=True, stop=True)
            q_s = SBA.tile([np_, 64], BF16, name=f"q_s{nm}")
            nc.vector.tensor_copy(q_s[:], q_ps[:])
            prod = SBA.tile([np_, T, 64], BF16, name=f"prod{nm}")
            nc.vector.tensor_tensor(
                prod[:], ctx_n[:],
                q_s[:].unsqueeze(1).broadcast_to([np_, T, 64]), ALU.mult)
            sc = SBA.tile([np_, T], F32, name=f"sc{nm}")
            nc.vector.tensor_reduce(sc[:], prod[:], AX.X, ALU.add)
            ex = SBA.tile([np_, T], BF16, name=f"ex{nm}")
            nc.scalar.activation(ex[:], sc[:], AF.Exp)
            bt = SBA.tile([np_, T], F32, name=f"bt{nm}")
            nc.scalar.activation(bt[:], delta[0:np_, :], AF.Exp, scale=nab_t)
            exbt = SBA.tile([np_, T], BF16, name=f"exbt{nm}")
            nc.vector.tensor_tensor(exbt[:], ex[:], bt[:], ALU.mult)
            den = SBA.tile([np_, 1], F32, name=f"den{nm}")
            with nc.allow_low_precision(reason="sum of 8 bf16 exps"):
                nc.vector.tensor_reduce(den[:], ex[:], AX.X, ALU.add)
            rcp = SBA.tile([np_, 1], F32, name=f"rcp{nm}")
            nc.vector.reciprocal(rcp[:], den[:])
            P_t = SBA.tile([np_, T, 64], BF16, name=f"P_t{nm}")
            nc.vector.tensor_tensor(
                P_t[:], ctx_n[:],
                ex[:].unsqueeze(2).broadcast_to([np_, T, 64]), ALU.mult)
            G_t = SBA.tile([np_, T, 64], BF16, name=f"G_t{nm}")
            nc.vector.tensor_tensor(
                G_t[:], ctx_n[:],
                exbt[:].unsqueeze(2).broadcast_to([np_, T, 64]), ALU.mult)
            t2_t = SBA.tile([np_, T, 64], BF16, name=f"t2_t{nm}")
            nc.vector.tensor_scalar(t2_t[:], G_t[:], ae_t, 0.0,
                                    ALU.mult, ALU.max)
            sm = SBA.tile([np_, T, 64], BF16, name=f"sm{nm}")
            nc.vector.tensor_tensor(sm[:], P_t[:], t2_t[:], ALU.add)
            mix0 = SBA.tile([np_, 64], F32, name=f"mix0{nm}")
            nc.vector.tensor_reduce(
                mix0[:], sm[:].rearrange("p t h -> p h t"), AX.X, ALU.add)
            mixs = SBA.tile([np_, 64], BF16, name=f"mixs{nm}")
            nc.vector.tensor_scalar(mixs[:], mix0[:], rcp[:], None, ALU.mult)
            mtr = PSA.tile([64, np_], BF16, name=f"mtr{nm}", tag=f"mtr{nm}")
            nc.tensor.transpose(mtr[:], mixs[:], identB[0:np_, 0:np_])
            nc.scalar.activation(combT[0:64, csl], mtr[:], AF.Copy)

        outT_ps = PSA.tile([64, SL], F32, name="outT_ps", tag="outT")
        nc.tensor.matmul(outT_ps[:], Wout, combT[:], start=True, stop=True)
        outT = SBA.tile([64, SL], BF16, name="outT")
        nc.scalar.activation(outT[:], outT_ps[:], AF.Tanh)

        # local x @ theta1 (node-major) then allgather it
        xq_ps = PSA.tile([128, 2, 64], F32, name="xq_ps", tag="xq")
        nc.tensor.matmul(xq_ps[:, 0, :], outT[:, 0:128], theta1,
                         start=True, stop=True)
        nc.tensor.matmul(xq_ps[0:16, 1, :], outT[:, 128:SL], theta1,
                         start=True, stop=True)
        xqA = WK.tile([128, 64], BF16, name="xqA_hold")
        nc.vector.tensor_copy(xqA[:], xq_ps[:, 0, :])
        xqB = SBA.tile([16, 64], BF16, name="xqB")
        nc.scalar.activation(xqB[:], xq_ps[0:16, 1, :], AF.Copy)
        nc.sync.dma_start(cc1_in[0:128, :], xqA[:])
        nc.scalar.dma_start(cc1_in[128:SL, :], xqB[:])
        nc.gpsimd.collective_compute(
            "AllGather", ALU.bypass, replica_groups=groups,
            ins=[cc1_in[:].opt()], outs=[cc1_out[:].opt()])
        nc.sync.dma_start(
            xq1[:, 0:5, :], cc1_out[0:640, :].rearrange("(k p) h -> p k h",
                                                        p=128))
        nc.scalar.dma_start(
            xq1[:, 5:NCH, :], cc1_out[640:NP, :].rearrange("(k p) h -> p k h",
                                                           p=128))
        if DEBUG:
            nc.gpsimd.dma_start(A["dbg_att"], outT[:])
            nc.gpsimd.dma_start(
                A["dbg_xq1"], xq1[:].rearrange("p k h -> p (k h)"))

    # keep the PE p-state ramped through cc1 with junk matmuls (gated on the
    # pre-collective attention output so they run during the collective)
    with tc.tile_pool(name="ps_warm", bufs=1, space="PSUM") as PSW:
        wps = PSW.tile([64, 512], F32, name="wps", tag="warm")
        for _ in range(90):
            nc.tensor.matmul(wps[:], xqA[:, 0:64], AT_L[:, 0, 0:512],
                             start=True, stop=True)
        for _ in range(42):
            nc.tensor.matmul(wps[:, 0:128], xqA[:, 0:64], AT_L[:, 0, 0:128],
                             start=True, stop=True)

    # ======================= hypergraph convs =======================
    # PSUM budget (8 banks): slabA x2 bufs (1 bank each), slabB x2 bufs,
    # xp (2 banks), glob-stage1 (1), xg packed (1).
    with tc.tile_pool(name="ps_slab", bufs=2, space="PSUM") as PSL, \
         tc.tile_pool(name="ps_aux", bufs=1, space="PSUM") as PSX, \
         tc.tile_pool(name="sb_conv", bufs=2) as SBC:

        x1T = WK.tile([64, NP], BF16, name="x1T")

        # conv1 local: A_c @ (x theta1) slab-by-slab, epilogue pipelined
        for si, (o, w) in enumerate(CH3):
            ps = PSL.tile([64, w], F32, name=f"psA{si}", tag="slabA")
            for k in range(NCH):
                nc.tensor.matmul(ps[:], xq1[:, k, :], AT_L[:, k, o:o + w],
                                 start=(k == 0), stop=(k == NCH - 1))
            l2 = SBC.tile([64, w], F32, name=f"l2a{si}", tag="lkA")
            nc.scalar.activation(l2[:], ps[:], AF.Identity, scale=0.2,
                                 bias=b1q)
            nc.vector.scalar_tensor_tensor(x1T[:, o:o + w], ps[:], b1c, l2[:],
                                           ALU.add, ALU.max)

        # global stage 1 (this core's rows only)
        psG1 = PSX.tile([64, SL], F32, name="psG1", tag="g1")
        for k in range(NCH):
            nc.tensor.matmul(psG1[:], xq1[:, k, :], AGc[:, k, :],
                             start=(k == 0), stop=(k == NCH - 1))
        l2g = SBC.tile([64, SL], F32, name="l2g", tag="lkg")
        nc.scalar.activation(l2g[:], psG1[:], AF.Identity, scale=0.2, bias=b1q)
        x1gT = SBC.tile([64, SL], BF16, name="x1gT", tag="x1g")
        nc.vector.scalar_tensor_tensor(x1gT[:], psG1[:], b1c, l2g[:],
                                       ALU.add, ALU.max)

        # x1 @ theta2 node-major (chunk k needs only its x1T slab)
        xp_ps = PSX.tile([128, NCH * 64], F32, name="xp_ps", tag="xp")
        for k in range(NCH):
            nc.tensor.matmul(xp_ps[:, k * 64:(k + 1) * 64],
                             x1T[:, k * 128:(k + 1) * 128], theta2,
                             start=True, stop=True)
        xq2 = SBC.tile([128, NCH, 64], BF16, name="xq2", tag="xq2")
        nc.scalar.activation(
            xq2[:], xp_ps[:].rearrange("p (k h) -> p k h", k=NCH), AF.Copy)

        # x1g @ theta2 (both row blocks packed into one psum bank)
        xg_ps = PSX.tile([128, 2, 64], F32, name="xg_ps", tag="xg")
        nc.tensor.matmul(xg_ps[:, 0, :], x1gT[:, 0:128], theta2,
                         start=True, stop=True)
        nc.tensor.matmul(xg_ps[0:16, 1, :], x1gT[:, 128:SL], theta2,
                         start=True, stop=True)
        xgA = SBC.tile([128, 64], BF16, name="xgA", tag="xgA")
        nc.vector.tensor_copy(xgA[:], xg_ps[:, 0, :])
        xgB = SBC.tile([16, 64], BF16, name="xgB", tag="xgB")
        nc.vector.tensor_copy(xgB[:], xg_ps[0:16, 1, :])

        # global partial2 slabs -> pC -> C-slot DMA (overlaps conv2)
        for si, (o, w) in enumerate(CH3):
            ps = PSL.tile([64, w], F32, name=f"psP{si}", tag="slabA")
            nc.tensor.matmul(ps[:], xgA[:], AGrA[:, o:o + w],
                             start=True, stop=False)
            nc.tensor.matmul(ps[:], xgB[:], AGrB[:, o:o + w],
                             start=False, stop=True)
            nc.scalar.activation(pC[:, o:o + w], ps[:], AF.Copy)
        nc.sync.dma_start(
            cc2_in[:, :, PAY_C:PAY_C + SL].rearrange("d p n -> p d n"),
            pC[:].rearrange("p (d n) -> p d n", d=NCORES))

        # conv2 local, 144-aligned slabs -> masked diff-slot writes
        # (every core writes both slots; masks make non-members contribute 0)
        SCH2 = ((0, 432), (432, 432), (864, 288))
        Sparts = []
        x2b = WK.tile([64, NP], F32, name="x2b")
        for si, (o, w) in enumerate(SCH2):
            ps = PSL.tile([64, w], F32, name=f"psB{si}", tag="slabB")
            for k in range(NCH):
                nc.tensor.matmul(ps[:], xq2[:, k, :], AT_L[:, k, o:o + w],
                                 start=(k == 0), stop=(k == NCH - 1))
            l2 = SBC.tile([64, w], F32, name=f"l2b{si}", tag="lkB")
            nc.scalar.activation(l2[:], ps[:], AF.Identity, scale=0.2,
                                 bias=b2q)
            pos = SBC.tile([64, w], F32, name=f"pos{si}", tag="lkC")
            nc.vector.scalar_tensor_tensor(pos[:], ps[:], b2c, l2[:],
                                           ALU.add, ALU.max)
            nc.vector.tensor_scalar(x2pay[:, o:o + w], pos[:], sg0c, None,
                                    ALU.mult)
            nc.gpsimd.tensor_scalar(x2b[:, o:o + w], pos[:], sg1c, None,
                                    ALU.mult)
            hi = min(o + w, N)
            if hi > o:
                sp = SBC.tile([64, 1], F32, name=f"Sp{si}", tag=f"Sp{si}")
                nc.vector.tensor_reduce(sp[:], pos[:, 0:hi - o], AX.X,
                                        ALU.add)
                Sparts.append(sp)
            d0, nd = o // SL, w // SL
            nc.sync.dma_start(
                cc2_in[d0:d0 + nd, :, 0:SL].rearrange("d p n -> p d n"),
                x2pay[:, o:o + w].rearrange("p (d n) -> p d n", d=nd))
            nc.scalar.dma_start(
                cc2_in[d0:d0 + nd, :, SL:2 * SL].rearrange("d p n -> p d n"),
                x2b[:, o:o + w].rearrange("p (d n) -> p d n", d=nd))

        # S_t (signed-signed cancels) -> signed one-hot column of S8
        S01 = SBC.tile([64, 1], F32, name="S01")
        nc.gpsimd.tensor_tensor(S01[:], Sparts[0][:], Sparts[1][:], ALU.add)
        S_col = SBC.tile([64, 1], F32, name="S_col")
        nc.gpsimd.tensor_tensor(S_col[:], S01[:], Sparts[2][:], ALU.add)
        nc.vector.tensor_tensor(
            S8[:], S_col[:].broadcast_to([64, 8]), sgn8, ALU.mult)  # plain onehot
        nc.sync.dma_start(
            cc2_in[:, :, PAY_S:PAY_S + 8].rearrange("d p f -> p d f"),
            S8[:].unsqueeze(1).broadcast_to([64, NCORES, 8]))

        if DEBUG:
            nc.gpsimd.dma_start(A["dbg_x2"], x2pay[:])
            nc.scalar.dma_start(
                A["dbg_pay"].rearrange("p (c f) -> p c f", c=NCORES),
                cc2_in[:].rearrange("c p f -> p c f"))
            nc.gpsimd.dma_start(A["dbg_x1g"], x1gT[:])
            nc.gpsimd.dma_start(A["dbg_xgA"], xgA[:])
            nc.scalar.dma_start(A["dbg_pC"], pC[:])

        # ---- collective 2: ReduceScatter ----
        nc.gpsimd.collective_compute(
            "ReduceScatter", ALU.add, replica_groups=groups,
            ins=[cc2_in[:].opt()], outs=[cc2_out[:].opt()])
        nc.sync.dma_start(R[:, PAY_S:PAY_S + 8], cc2_out[:, PAY_S:PAY_S + 8])
        nc.scalar.dma_start(R[:, 0:PAY_S], cc2_out[:, 0:PAY_S])
        if DEBUG:
            nc.scalar.dma_start(A["dbg_R"], R[:])

    # ======================= final stage =======================
    if True:
        with tc.tile_pool(name="sb_fin", bufs=1) as SBF, \
             tc.tile_pool(name="ps_fin", bufs=1, space="PSUM") as PSF:
            # S column vector: S[t] = sum_p slotS[p, t]
            Sc_ps = PSF.tile([8, 1], F32, name="Sc_ps", tag="f")
            nc.tensor.matmul(Sc_ps[:], R[:, PAY_S:PAY_S + 8], ones64,
                             start=True, stop=True)
            S_cv = SBF.tile([8, 1], F32, name="S_cv")
            nc.vector.tensor_copy(S_cv[:], Sc_ps[:])
            # y = leaky(M @ S) (difference matrix folded into M on host)
            y_ps = PSF.tile([64, 1], F32, name="y_ps", tag="f2")
            nc.tensor.matmul(y_ps[:], cMT[:], S_cv[:], start=True, stop=True)
            y1 = SBF.tile([64, 1], F32, name="y1")
            nc.vector.tensor_scalar(y1[:], y_ps[:], 0.2, None, ALU.mult)
            y_s = SBF.tile([64, 1], F32, name="y_s")
            nc.vector.tensor_tensor(y_s[:], y_ps[:], y1[:], ALU.max)
            # wat row = (w2 @ y)^T; softmax (values are O(1), skip max-sub)
            wat_ps = PSF.tile([1, 7], F32, name="wat_ps", tag="f3")
            nc.tensor.matmul(wat_ps[:], y_s[:], cw2T[:], start=True, stop=True)
            exw = SBF.tile([1, 7], F32, name="exw")
            nc.scalar.activation(exw[:], wat_ps[:], AF.Exp)
            denw = SBF.tile([1, 1], F32, name="denw")
            nc.vector.tensor_reduce(denw[:], exw[:], AX.X, ALU.add)
            rw = SBF.tile([1, 1], F32, name="rw")
            nc.vector.reciprocal(rw[:], denw[:])

            # head decomposed so no weight broadcast is needed:
            # res = leaky(Wl_xg.xg + w0 (Wl_xx.sub0) + w2 (Wl_xx.sub2) + bl)
            R0b = SBF.tile([64, SL], BF16, name="R0b")
            nc.vector.tensor_copy(R0b[:], R[:, 0:SL])
            R1b = SBF.tile([64, SL], BF16, name="R1b")
            nc.vector.tensor_copy(R1b[:], R[:, SL:2 * SL])
            h3_ps = PSF.tile([1, 3, SL], F32, name="h3_ps", tag="f4")
            nc.tensor.matmul(h3_ps[:, 1, :], Wl_xx_b, R0b[:],
                             start=True, stop=True)
            nc.tensor.matmul(h3_ps[:, 2, :], Wl_xx_b, R1b[:],
                             start=True, stop=True)
            xg = SBF.tile([64, SL], BF16, name="xg")
            lxg = SBF.tile([64, SL], F32, name="lxg")
            nc.scalar.activation(lxg[:], R[:, PAY_C:PAY_C + SL], AF.Identity,
                                 scale=0.2, bias=b2q)
            nc.vector.scalar_tensor_tensor(xg[:], R[:, PAY_C:PAY_C + SL],
                                           b2c, lxg[:], ALU.add, ALU.max)
            nc.tensor.matmul(h3_ps[:, 0, :], Wl_xg_b, xg[:],
                             start=True, stop=True)
            a0 = SBF.tile([1, SL], F32, name="a0")
            nc.vector.tensor_scalar(a0[:], h3_ps[:, 1, :], exw[0:1, 0:1],
                                    None, ALU.mult)
            a1 = SBF.tile([1, SL], F32, name="a1")
            nc.vector.scalar_tensor_tensor(a1[:], h3_ps[:, 2, :],
                                           exw[0:1, 2:3], a0[:], ALU.mult,
                                           ALU.add)
            pre = SBF.tile([1, SL], F32, name="pre")
            nc.vector.scalar_tensor_tensor(pre[:], a1[:], rw[0:1, 0:1],
                                           h3_ps[:, 0, :], ALU.mult, ALU.add)
            r2 = SBF.tile([1, SL], F32, name="r2")
            nc.vector.tensor_scalar(r2[:], pre[:], 0.2, cbl[0:1, 1:2],
                                    ALU.mult, ALU.add)
            res_s = SBF.tile([1, SL], F32, name="res_s")
            nc.vector.scalar_tensor_tensor(res_s[:], pre[:], cbl[0:1, 0:1],
                                           r2[:], ALU.add, ALU.max)
            nc.sync.dma_start(out_ap[:], res_s[:])

    stack.close()


# --------------------------------------------------------------------------
# entry points
# --------------------------------------------------------------------------

def _make_nc():
    if "nc" in _NC_CACHE:
        return _NC_CACHE["nc"]
    import concourse.bacc as bacc
    import concourse.mybir as mybir
    from concourse import tile

    nc = bacc.Bacc("TRN2", target_bir_lowering=False, debug=False,
                   enable_asserts=True, num_devices=NCORES)
    DT = {"f32": mybir.dt.float32, "bf16": mybir.dt.bfloat16,
          "f32r": mybir.dt.float32r}
    A = {}
    for nm, shape, dt_ in _IN_SPECS:
        A[nm] = nc.dram_tensor(nm, list(shape), DT[dt_],
                               kind="ExternalInput").ap()
    out_h = nc.dram_tensor("out_part", [1, SL], mybir.dt.float32,
                           kind="ExternalOutput")
    if DEBUG:
        for nm, shp in (("dbg_att", [64, SL]), ("dbg_xq1", [128, NCH * 64]),
                        ("dbg_x2", [64, NP]), ("dbg_R", [64, PAYW]),
                        ("dbg_pay", [64, NCORES * PAYW]),
                        ("dbg_x1g", [64, SL]), ("dbg_xgA", [128, 64]),
                        ("dbg_pC", [64, NP])):
            A[nm] = nc.dram_tensor(nm, shp, mybir.dt.float32,
                                   kind="ExternalOutput").ap()
    with tile.TileContext(nc) as tc:
        build_program(tc, A, out_h.ap())
    nc.compile()
    _NC_CACHE["nc"] = nc
    return nc


def kernel(**inputs):
    from concourse.bass_utils import run_bass_kernel_spmd
    nc = _make_nc()
    in_maps = _host_prep(inputs)
    res = run_bass_kernel_spmd(nc, in_maps, list(range(NCORES)))
    full = np.concatenate(
        [np.asarray(res.results[c]["out_part"])[0] for c in range(NCORES)])
    return np.ascontiguousarray(full[:N, None].astype(np.float32))
